# revision 19
# baseline (speedup 1.0000x reference)
"""Trainium2 Bass kernel for nn_Encoder_47167330845225.

Three embedding+LSTM encoders (source-comment, commit-msg, issue) + scalar
merge + final projection.  Data-parallel over the PR batch (B=64) across 8
NeuronCores; embedding tables and weights replicated.

v2 design (vs v1 which precomputed X = emb@WihT through DRAM):
  - x-projection fused into the recurrence PSUM: gathered embedding chunks
    live in SBUF and Wih matmuls accumulate into the same z banks as the
    recurrence.  No X DRAM bounce, no phase-A PSUM->SBUF copy traffic.
  - recurrence weights in split-fp8 (DoubleRow perf mode, 2 fp8 values per
    PE row):  Whh ~= fp8(64*W)/64 + fp8(1024*(W - hi/64))/1024, consumed
    against two fp8 copies of the state (32*h and 2*h).  All products land
    at a common 2048*z scale in PSUM; Wih is pre-scaled by 2048 in bf16 so
    the x-path accumulates in the same domain.  ACT dequantizes with
    scale=1/2048.  Measured end-to-end ~0.7% rel err vs fp32 reference.
  - gate order per step is bank-major [g | i..f | f..o] in 3 PSUM banks so
    one strided ACT does all 12 sigmoid m-tiles, one ACT does tanh(g)
    (Sigmoid+Tanh live in the same ACT LUT set - no table swaps), one ACT
    does tanh(c).
  - steady state interleaves sc+cm (+is every other round) so the tensor
    engine never waits on the gate chain; after cm ends, sc is split into
    two 40-wide half-streams to keep two streams in flight.
  - gathers run on 4 SWDGE queues with multi-chunk prefetch; Pool engine
    does nothing else so descriptor generation never blocks elementwise.
"""

import math
import os

import numpy as np
import ml_dtypes

BF16 = ml_dtypes.bfloat16
FP8 = ml_dtypes.float8_e4m3          # dt.float8e4
P = 128
V, H, E = 32000, 512, 256
G = 4 * H                            # 2048 gate rows
B, NCOM, LSC, LCM, LIS = 64, 10, 128, 64, 32
NCORES = 8
BPC = B // NCORES                    # 8 PRs per core
NSEQ = BPC * NCOM                    # 80 commit sequences per core
MT = G // P                          # 16 m-tiles
KH = H // P                          # 4 k-tiles over H
KE = E // P                          # 2 k-tiles over E

# (name, T, Nb, chunk_steps, n_chunk_bufs, gather queues)
# chunk_steps * Nb is capped at 640 tokens per dma_gather: 1280-token
# gathers blow the SWDGE descriptor ring and wedge the exec unit
# (NRT_EXEC_UNIT_UNRECOVERABLE) even though CoreSim accepts them.
CHAINS = {
    "sc": (LSC, NSEQ, 8, 8, (0,)),
    "cm": (LCM, NSEQ, 8, 6, (0,)),
    "is": (LIS, BPC, 32, 1, (0,)),
}
_DEBUG = int(os.environ.get("BASSK_DEBUG", "0"))
# bisection flags: NOFP8 -> bf16 recurrence; SPLITACT -> per-bank sigmoid ACTs
_NOFP8 = int(os.environ.get("BASSK_NOFP8", "0"))
_SPLITACT = int(os.environ.get("BASSK_SPLITACT", "0"))
_NOIS = int(os.environ.get("BASSK_NOIS", "0"))      # skip is-chain (runtime probe)
_NOTAIL = int(os.environ.get("BASSK_NOTAIL", "0"))  # skip phase-2 (runtime probe)
_S8 = int(os.environ.get("BASSK_S8", "0"))          # 640-token gather chunks
_NOTANH = int(os.environ.get("BASSK_NOTANH", "0"))  # tanh via 2*sigmoid(2x)-1

# bank-major gate order g,i,f,o (pytorch order is i,f,g,o).  g first so
# tanh(g) can be computed while the sigmoid banks' matmuls still run.
_GPERM = np.r_[2 * H:3 * H, 0:H, H:2 * H, 3 * H:4 * H]

_CACHE = {}


class _Layout:
    """Placement of the 16 z m-tiles into PSUM banks for one stream."""

    def __init__(self, tag, w, nbanks, bank_ms):
        self.tag = tag
        self.w = w
        self.nbanks = nbanks
        self.bank_ms = bank_ms          # bank -> list of m-tile indices
        self.col = {}                   # m -> (bank, col0)
        for b, ms in enumerate(bank_ms):
            for i, m in enumerate(ms):
                self.col[m] = (b, i * w)


def _mk_layouts():
    # phase1: 3 banks. bank0: g(m0-3); bank1: m4-9 (i + f/2); bank2: m10-15.
    l_a = _Layout("zA", NSEQ, 3, [[0, 1, 2, 3], [4, 5, 6, 7, 8, 9],
                                  [10, 11, 12, 13, 14, 15]])
    l_b = _Layout("zB", NSEQ, 3, [[0, 1, 2, 3], [4, 5, 6, 7, 8, 9],
                                  [10, 11, 12, 13, 14, 15]])
    # is: everything in one bank (16 m-tiles x 8 cols = 128 <= 512)
    l_is = _Layout("zIS", BPC, 1, [list(range(16))])
    # tail half-streams: 2 banks. bank0: g; bank1: i,f,o (12 x 40 = 480)
    l_t0 = _Layout("zT0", NSEQ // 2, 2, [[0, 1, 2, 3], list(range(4, 16))])
    l_t1 = _Layout("zT1", NSEQ // 2, 2, [[0, 1, 2, 3], list(range(4, 16))])
    return l_a, l_b, l_is, l_t0, l_t1


def _emit(tc, dram, scratch, has_bias):
    import concourse.bass as bass
    import concourse.mybir as mybir
    from contextlib import ExitStack

    dt = mybir.dt
    A = mybir.ActivationFunctionType
    OP = mybir.AluOpType
    DR = mybir.MatmulPerfMode.DoubleRow
    nc = tc.nc
    INV = 1.0 / 2048.0

    with ExitStack() as ctx:
        const = ctx.enter_context(tc.tile_pool(name="const", bufs=1))

        # ---- persistent SBUF: weights, indices, states ----
        whi_sb, wlo_sb, wih_sb, idx_sb, whh_sb = {}, {}, {}, {}, {}
        h_sb, c_sb, h32_sb, h2_sb, b_sb = {}, {}, {}, {}, {}
        for name, (T, Nb, S, NBUF, QS) in CHAINS.items():
            if _NOFP8:
                whh = const.tile([P, KH, G], dt.bfloat16, tag=f"whh_{name}")
                nc.sync.dma_start(whh[:], dram[f"whh_{name}"].rearrange("(k p) g -> p k g", p=P))
                whh_sb[name] = whh
            else:
                whi = const.tile([P, KH, G], dt.float8e4, tag=f"whi_{name}")
                nc.sync.dma_start(whi[:], dram[f"whi_{name}"].rearrange("(k p) g -> p k g", p=P))
                whi_sb[name] = whi
                wlo = const.tile([P, KH, G], dt.float8e4, tag=f"wlo_{name}")
                nc.sync.dma_start(wlo[:], dram[f"wlo_{name}"].rearrange("(k p) g -> p k g", p=P))
                wlo_sb[name] = wlo
            wih = const.tile([P, KE, G], dt.bfloat16, tag=f"wih_{name}")
            nc.sync.dma_start(wih[:], dram[f"wih_{name}"].rearrange("(k p) g -> p k g", p=P))
            wih_sb[name] = wih
            ntok = T * Nb
            ix = const.tile([P, ntok // 16], dt.int16, tag=f"idx_{name}")
            nc.sync.dma_start(ix[:], dram[f"idx_{name}"])
            idx_sb[name] = ix
            if has_bias:
                bb = const.tile([1, G], dt.float32, tag=f"bias_{name}")
                nc.sync.dma_start(bb[:], dram[f"bias_{name}"])
                b_sb[name] = bb
            h = const.tile([P, KH, Nb], dt.bfloat16, tag=f"h_{name}")
            nc.vector.memset(h[:], 0.0)
            h_sb[name] = h
            c = const.tile([P, KH, Nb], dt.bfloat16, tag=f"c_{name}")
            nc.vector.memset(c[:], 0.0)
            c_sb[name] = c
            h32 = const.tile([P, KH, Nb], dt.float8e4, tag=f"h32_{name}")
            nc.vector.memset(h32[:], 0.0)
            h32_sb[name] = h32
            h2 = const.tile([P, KH, Nb], dt.float8e4, tag=f"h2_{name}")
            nc.vector.memset(h2[:], 0.0)
            h2_sb[name] = h2
        ones_sb = None
        if has_bias:
            ones_sb = const.tile([1, NSEQ], dt.bfloat16, tag="ones")
            nc.vector.memset(ones_sb[:], 1.0)

        wm_sb = const.tile([P, KH, 4], dt.bfloat16, tag="wm")
        nc.sync.dma_start(wm_sb[:], dram["wm"].rearrange("(k p) c -> p k c", p=P))
        bm_sb = const.tile([1, 2], dt.float32, tag="bm")
        nc.sync.dma_start(bm_sb[:], dram["bm"])
        wfm_sb = const.tile([P, 2, H], dt.bfloat16, tag="wfm")
        nc.sync.dma_start(wfm_sb[:], dram["wf_m"].rearrange("c p m -> p c m"))
        wfh_sb = const.tile([P, 2, KH, H], dt.bfloat16, tag="wfh")
        nc.sync.dma_start(wfh_sb[:], dram["wf_h"].rearrange("c (k p) m -> p c k m", p=P))
        bf_sb = const.tile([P, KH, 2], dt.float32, tag="bf")
        nc.sync.dma_start(bf_sb[:], dram["bf"].rearrange("(m p) c -> p m c", p=P))

        # ---- gather machinery: chunk ci of chain covers steps
        # [ci*S, (ci+1)*S); tile slot rotates through NBUF buffers ----
        chunk_pool = ctx.enter_context(tc.tile_pool(name="chunks", bufs=1))
        chunk_tiles = {name: {} for name in CHAINS}

        def emit_gather(name, ci):
            T, Nb, S, NBUF, QS = CHAINS[name]
            gch = S * Nb
            emb = chunk_pool.tile([P, KE, gch], dt.bfloat16,
                                  tag=f"emb_{name}", bufs=NBUF,
                                  name=f"emb_{name}_{ci}")
            chunk_tiles[name][ci] = emb
            nc.gpsimd.dma_gather(
                out_ap=emb[:],
                in_ap=dram[f"tab_{name}"][:, :],
                idxs_ap=idx_sb[name][:, ci * (gch // 16):(ci + 1) * (gch // 16)],
                num_idxs=gch,
                num_idxs_reg=gch,
                elem_size=E,
                transpose=True,
                queue_num=0,
            )

        # prologue prefetch: first NBUF chunks of each chain, interleaved so
        # the chunks needed first are gathered first.
        prologue = []
        for i in range(max(nb for (_, _, _, nb, _) in CHAINS.values())):
            for name, (T, Nb, S, NBUF, QS) in CHAINS.items():
                nchunks = (T + S - 1) // S
                if i < min(NBUF, nchunks):
                    prologue.append((name, i))
        for name, ci in prologue:
            emit_gather(name, ci)

        def maybe_refill(r):
            # emit gather for chunk ci of sc once its buffer slot frees:
            # slot (ci mod NBUF) is free after round (ci-NBUF+1)*S - 1.
            for name, (T, Nb, S, NBUF, QS) in CHAINS.items():
                nchunks = (T + S - 1) // S
                for ci in range(NBUF, nchunks):
                    if (ci - NBUF + 1) * S == r + 1:
                        emit_gather(name, ci)

        gate_pool = ctx.enter_context(tc.tile_pool(name="gates", bufs=2))

        def step(zpool, L, name, t, j0):
            """One LSTM step for columns [j0, j0+L.w) of chain `name`."""
            T, Nb, S, NBUF, QS = CHAINS[name]
            w = L.w
            z = zpool.tile([P, L.nbanks, 512], dt.float32, tag=L.tag)
            emb = chunk_tiles[name][t // S]
            xoff = (t % S) * Nb + j0
            wih = wih_sb[name]
            h32 = h32_sb[name]
            h2 = h2_sb[name]

            rec_mm = KH if _NOFP8 else 4                 # recurrence mms per m
            for b in range(L.nbanks):
                ms = L.bank_ms[b]
                n_mm = len(ms) * (rec_mm + KE + (1 if has_bias else 0))
                i_mm = 0
                for kind in range(4 if has_bias else 3):
                    for m in ms:
                        _, col = L.col[m]
                        out = z[:, b, col:col + w]
                        if kind == 0:
                            if _NOFP8:                   # bf16 recurrence
                                for k in range(KH):
                                    nc.tensor.matmul(
                                        out, lhsT=whh_sb[name][:, k, m * P:(m + 1) * P],
                                        rhs=h_sb[name][:, k, j0:j0 + w],
                                        start=(i_mm == 0), stop=(i_mm == n_mm - 1),
                                        skip_group_check=True)
                                    i_mm += 1
                            else:                        # hi fp8 DoubleRow
                                for kp in range(KH // 2):
                                    nc.tensor.matmul(
                                        out, lhsT=whi_sb[name][:, 2 * kp:2 * kp + 2, m * P:(m + 1) * P],
                                        rhs=h32[:, 2 * kp:2 * kp + 2, j0:j0 + w],
                                        perf_mode=DR,
                                        start=(i_mm == 0), stop=(i_mm == n_mm - 1),
                                        skip_group_check=True)
                                    i_mm += 1
                        elif kind == 1:                  # lo fp8 DoubleRow
                            if _NOFP8:
                                continue
                            for kp in range(KH // 2):
                                nc.tensor.matmul(
                                    out, lhsT=wlo_sb[name][:, 2 * kp:2 * kp + 2, m * P:(m + 1) * P],
                                    rhs=h2[:, 2 * kp:2 * kp + 2, j0:j0 + w],
                                    perf_mode=DR,
                                    start=(i_mm == 0), stop=(i_mm == n_mm - 1),
                                    skip_group_check=True)
                                i_mm += 1
                        elif kind == 2:                  # x-path bf16
                            for k in range(KE):
                                nc.tensor.matmul(
                                    out, lhsT=wih[:, k, m * P:(m + 1) * P],
                                    rhs=emb[:, k, xoff:xoff + w],
                                    start=(i_mm == 0), stop=(i_mm == n_mm - 1),
                                    skip_group_check=True)
                                i_mm += 1
                        else:                            # bias (K=1 matmul)
                            nc.tensor.matmul(
                                out, lhsT=b_sb[name][0:1, m * P:(m + 1) * P],
                                rhs=ones_sb[0:1, :w],
                                start=(i_mm == 0), stop=(i_mm == n_mm - 1),
                                skip_group_check=True)
                            i_mm += 1
                if b == 0:
                    # tanh(g) straight out of the g bank, early.
                    gg = gate_pool.tile([P, KH, w], dt.bfloat16, tag=f"gg_{L.tag}")
                    gin = z[:, 0, 0:4 * w].rearrange("p (k j) -> p k j", j=w)
                    if _NOTANH:
                        g32 = gate_pool.tile([P, KH, w], dt.float32, tag=f"g32_{L.tag}")
                        nc.scalar.activation(g32[:], gin, A.Sigmoid, scale=2.0 * INV)
                        nc.vector.tensor_scalar(gg[:], g32[:], 2.0, -1.0, OP.mult, OP.add)
                    else:
                        nc.scalar.activation(gg[:], gin, A.Tanh, scale=INV)

            # sigmoid over the i/f/o banks in one strided ACT.
            nsig = 12 * w
            sig = gate_pool.tile([P, nsig], dt.bfloat16, tag=f"sig_{L.tag}")
            if L.nbanks == 3 and _SPLITACT:
                nc.scalar.activation(sig[:, 0:6 * w], z[:, 1, 0:6 * w], A.Sigmoid, scale=INV)
                nc.scalar.activation(sig[:, 6 * w:12 * w], z[:, 2, 0:6 * w], A.Sigmoid, scale=INV)
            elif L.nbanks == 3:
                nc.scalar.activation(
                    sig[:].rearrange("p (a b) -> p a b", a=2),
                    z[:, 1:3, 0:6 * w], A.Sigmoid, scale=INV)
            elif L.nbanks == 2:
                nc.scalar.activation(sig[:], z[:, 1, 0:nsig], A.Sigmoid, scale=INV)
            else:
                nc.scalar.activation(sig[:], z[:, 0, 4 * w:16 * w], A.Sigmoid, scale=INV)
            ig = sig[:, 0 * 4 * w:1 * 4 * w].rearrange("p (k j) -> p k j", j=w)
            fg = sig[:, 1 * 4 * w:2 * 4 * w].rearrange("p (k j) -> p k j", j=w)
            og = sig[:, 2 * 4 * w:3 * 4 * w].rearrange("p (k j) -> p k j", j=w)

            cs = c_sb[name][:, :, j0:j0 + w]
            hs = h_sb[name][:, :, j0:j0 + w]
            tmp = gate_pool.tile([P, KH, w], dt.bfloat16, tag=f"tmp_{L.tag}")
            tch = gate_pool.tile([P, KH, w], dt.bfloat16, tag=f"tch_{L.tag}")
            nc.vector.tensor_mul(tmp[:], ig, gg[:])
            nc.vector.tensor_mul(cs, fg, cs)
            nc.vector.tensor_add(cs, cs, tmp[:])
            if _NOTANH:
                t32 = gate_pool.tile([P, KH, w], dt.float32, tag=f"t32_{L.tag}")
                nc.scalar.activation(t32[:], cs, A.Sigmoid, scale=2.0)
                nc.vector.tensor_scalar(tch[:], t32[:], 2.0, -1.0, OP.mult, OP.add)
            else:
                nc.scalar.activation(tch[:], cs, A.Tanh)
            nc.vector.tensor_mul(hs, og, tch[:])
            if not _NOFP8:
                nc.vector.tensor_scalar(h32_sb[name][:, :, j0:j0 + w], hs, 32.0, None, OP.mult)
                nc.vector.tensor_scalar(h2_sb[name][:, :, j0:j0 + w], hs, 2.0, None, OP.mult)

        l_a, l_b, l_is, l_t0, l_t1 = _mk_layouts()

        # ---- phase 1: rounds 0..63 -- sc + cm (+ is every other round) ----
        with tc.tile_pool(name="z1", bufs=1, space="PSUM") as zpool:
            for r in range(LCM):
                step(zpool, l_a, "sc", r, 0)
                if r % 2 == 0 and not _NOIS:
                    step(zpool, l_is, "is", r // 2, 0)
                step(zpool, l_b, "cm", r, 0)
                maybe_refill(r)

        # ---- phase 2: rounds 64..127 -- sc as two half-streams ----
        with tc.tile_pool(name="z2", bufs=1, space="PSUM") as zpool:
            for r in range(LCM, LSC):
                if _NOTAIL:
                    break
                step(zpool, l_t0, "sc", r, 0)
                step(zpool, l_t1, "sc", r, NSEQ // 2)
                maybe_refill(r)

        if _DEBUG:
            for name in CHAINS:
                nc.sync.dma_start(dram[f"dbg_h_{name}"][:], h_sb[name][:])
                nc.sync.dma_start(dram[f"dbg_c_{name}"][:], c_sb[name][:])

        # ---- phase 3: merge + final projection (same as v1) ----
        with tc.tile_pool(name="fin", bufs=1) as fin, \
             tc.tile_pool(name="fpsum", bufs=2, space="PSUM") as fp:
            for side, st1, st2, st_is in (
                    (0, h_sb["sc"], h_sb["cm"], h_sb["is"]),
                    (1, c_sb["sc"], c_sb["cm"], c_sb["is"])):
                # hm[j] = hcat[j] . wm  over both halves
                mm = fp.tile([1, NSEQ], dt.float32, tag="mg")
                for half, st in ((0, st1), (1, st2)):
                    for k in range(KH):
                        col = 2 * side + half
                        nc.tensor.matmul(
                            mm[:], lhsT=wm_sb[:, k, col:col + 1], rhs=st[:, k, :],
                            start=(half == 0 and k == 0),
                            stop=(half == 1 and k == KH - 1),
                            skip_group_check=True)
                hm_bf = fin.tile([1, NSEQ], dt.bfloat16, tag=f"hm{side}")
                nc.vector.tensor_scalar(
                    hm_bf[:], mm[:], bm_sb[0:1, side:side + 1], None, OP.add)
                # reshape [80] -> [10, 8] via DRAM bounce; zero-pad to 128 rows
                nc.sync.dma_start(scratch[side][None, :], hm_bf[0:1, :])
                hmT = fin.tile([P, BPC], dt.bfloat16, tag=f"hmT{side}")
                nc.vector.memset(hmT[:], 0.0)
                nc.sync.dma_start(
                    hmT[:NCOM, :], scratch[side].rearrange("(p n) -> n p", n=NCOM))
                out_sb = fin.tile([P, KH, BPC], dt.float32, tag=f"out{side}")
                for m in range(KH):
                    pf = fp.tile([P, BPC], dt.float32, tag="fin")
                    nc.tensor.matmul(
                        pf[:], lhsT=wfm_sb[:, side, m * P:(m + 1) * P], rhs=hmT[:],
                        start=True, stop=False, skip_group_check=True)
                    for k in range(KH):
                        nc.tensor.matmul(
                            pf[:], lhsT=wfh_sb[:, side, k, m * P:(m + 1) * P],
                            rhs=st_is[:, k, :],
                            start=False, stop=(k == KH - 1),
                            skip_group_check=True)
                    nc.scalar.activation(
                        out_sb[:, m, :], pf[:], A.Identity,
                        bias=bf_sb[:, m, side:side + 1])
                nc.sync.dma_start(dram["ho" if side == 0 else "co"][:], out_sb[:])


def _build(has_bias):
    import concourse.mybir as mybir
    import concourse.tile as tile
    from concourse import bacc

    dt = mybir.dt
    nc = bacc.Bacc("TRN2", target_bir_lowering=False, debug=False,
                   num_devices=NCORES)
    dram = {}
    for name, (T, Nb, S, NBUF, QS) in CHAINS.items():
        dram[f"tab_{name}"] = nc.dram_tensor(f"tab_{name}", [V, E], dt.bfloat16, kind="ExternalInput").ap()
        if _NOFP8:
            dram[f"whh_{name}"] = nc.dram_tensor(f"whh_{name}", [H, G], dt.bfloat16, kind="ExternalInput").ap()
        else:
            dram[f"whi_{name}"] = nc.dram_tensor(f"whi_{name}", [H, G], dt.float8e4, kind="ExternalInput").ap()
            dram[f"wlo_{name}"] = nc.dram_tensor(f"wlo_{name}", [H, G], dt.float8e4, kind="ExternalInput").ap()
        dram[f"wih_{name}"] = nc.dram_tensor(f"wih_{name}", [E, G], dt.bfloat16, kind="ExternalInput").ap()
        if has_bias:
            dram[f"bias_{name}"] = nc.dram_tensor(f"bias_{name}", [1, G], dt.float32, kind="ExternalInput").ap()
        dram[f"idx_{name}"] = nc.dram_tensor(f"idx_{name}", [P, T * Nb // 16], dt.int16, kind="ExternalInput").ap()
    dram["wm"] = nc.dram_tensor("wm", [H, 4], dt.bfloat16, kind="ExternalInput").ap()
    dram["bm"] = nc.dram_tensor("bm", [1, 2], dt.float32, kind="ExternalInput").ap()
    dram["wf_m"] = nc.dram_tensor("wf_m", [2, P, H], dt.bfloat16, kind="ExternalInput").ap()
    dram["wf_h"] = nc.dram_tensor("wf_h", [2, H, H], dt.bfloat16, kind="ExternalInput").ap()
    dram["bf"] = nc.dram_tensor("bf", [H, 2], dt.float32, kind="ExternalInput").ap()
    dram["ho"] = nc.dram_tensor("ho", [P, KH, BPC], dt.float32, kind="ExternalOutput").ap()
    dram["co"] = nc.dram_tensor("co", [P, KH, BPC], dt.float32, kind="ExternalOutput").ap()
    if _DEBUG:
        for name, (T, Nb, S, NBUF, QS) in CHAINS.items():
            dram[f"dbg_h_{name}"] = nc.dram_tensor(f"dbg_h_{name}", [P, KH, Nb], dt.bfloat16, kind="ExternalOutput").ap()
            dram[f"dbg_c_{name}"] = nc.dram_tensor(f"dbg_c_{name}", [P, KH, Nb], dt.bfloat16, kind="ExternalOutput").ap()

    scratch = [nc.dram_tensor(f"hmsc{i}", [NSEQ], dt.bfloat16, kind="Internal").ap() for i in range(2)]

    with tile.TileContext(nc) as tc:
        _emit(tc, dram, scratch, has_bias)
    nc.compile()
    return nc


def _prep_inputs(inputs):
    """Build the 8 per-core input maps from full-size inputs."""
    comments = np.asarray(inputs["comments"]).astype(np.int32)
    cm = np.asarray(inputs["cm"]).astype(np.int32)
    issue = np.asarray(inputs["issue"]).astype(np.int32)

    def bf(x):
        return np.ascontiguousarray(np.asarray(x).astype(BF16))

    has_bias = any(
        np.any(np.asarray(inputs[b]) != 0) for b in ("b_sc", "b_cm", "b_is"))

    shared = {}
    for name, src in (("sc", "emb_sc"), ("cm", "emb_cm"), ("is", "emb_is")):
        shared[f"tab_{name}"] = bf(inputs[src])
    for name, whh, wih, b in (("sc", "Whh_sc", "Wih_sc", "b_sc"),
                              ("cm", "Whh_cm", "Wih_cm", "b_cm"),
                              ("is", "Whh_is", "Wih_is", "b_is")):
        Wp = np.asarray(inputs[whh], np.float32)[_GPERM].T      # [H, G]
        hi = (64.0 * Wp).astype(FP8)
        resid = Wp - hi.astype(np.float32) / 64.0
        lo = (1024.0 * resid).astype(FP8)
        shared[f"whi_{name}"] = np.ascontiguousarray(hi)
        shared[f"wlo_{name}"] = np.ascontiguousarray(lo)
        shared[f"whh_{name}"] = bf(2048.0 * Wp)
        Up = np.asarray(inputs[wih], np.float32)[_GPERM].T      # [E, G]
        shared[f"wih_{name}"] = bf(2048.0 * Up)
        if has_bias:
            shared[f"bias_{name}"] = np.ascontiguousarray(
                2048.0 * np.asarray(inputs[b], np.float32)[_GPERM][None, :])
    wm = np.stack([np.asarray(inputs["Wmh"])[0, :H],
                   np.asarray(inputs["Wmh"])[0, H:],
                   np.asarray(inputs["Wmc"])[0, :H],
                   np.asarray(inputs["Wmc"])[0, H:]], axis=1)   # [H, 4]
    shared["wm"] = bf(wm)
    shared["bm"] = np.array([[inputs["bmh"][0], inputs["bmc"][0]]], dtype=np.float32)
    wf_m = np.zeros((2, P, H), np.float32)
    wf_h = np.zeros((2, H, H), np.float32)
    for i, w in enumerate(("Wfh", "Wfc")):
        WT = np.asarray(inputs[w]).T                    # [522, 512]
        wf_m[i, :NCOM] = WT[:NCOM]
        wf_h[i] = WT[NCOM:]
    shared["wf_m"] = bf(wf_m)
    shared["wf_h"] = bf(wf_h)
    shared["bf"] = np.ascontiguousarray(
        np.stack([inputs["bfh"], inputs["bfc"]], axis=1).astype(np.float32))

    def wrap16(flat):
        # dma_gather index layout: idx i -> [i % 16, i // 16], int16,
        # replicated over all 128 partitions (8 gpsimd channels x 16).
        w = flat.reshape(-1, 16).T.astype(np.int16)     # [16, n/16]
        return np.ascontiguousarray(np.tile(w, (P // 16, 1)))

    in_maps = []
    for c in range(NCORES):
        m = dict(shared)
        prs = slice(c * BPC, (c + 1) * BPC)
        # time-major token ids: token f = t*Nb + j, j = pr_local*NCOM + ncom
        sc = comments[prs].reshape(NSEQ, LSC)
        m["idx_sc"] = wrap16(sc.T.reshape(-1))
        cmv = cm[prs].reshape(NSEQ, LCM)
        m["idx_cm"] = wrap16(cmv.T.reshape(-1))
        isv = issue[prs]                                # [8, T]
        m["idx_is"] = wrap16(isv.T.reshape(-1))
        in_maps.append(m)
    return in_maps, has_bias


def kernel(**inputs):
    from concourse.bass_utils import run_bass_kernel_spmd

    in_maps, has_bias = _prep_inputs(inputs)
    if "nc" not in _CACHE:
        _CACHE["nc"] = _build(has_bias)
    res = run_bass_kernel_spmd(_CACHE["nc"], in_maps, core_ids=list(range(NCORES)))
    h = np.zeros((B, H), np.float32)
    c = np.zeros((B, H), np.float32)
    for ci, r in enumerate(res.results):
        # ho [128, 4, 8]: ho[p, k, j] = h[8*ci + j, 128*k + p]
        h[ci * BPC:(ci + 1) * BPC] = r["ho"].transpose(2, 1, 0).reshape(BPC, H)
        c[ci * BPC:(ci + 1) * BPC] = r["co"].transpose(2, 1, 0).reshape(BPC, H)
    return h[None], c[None]


# revision 21
# speedup vs baseline: 2.4085x; 2.4085x over previous
"""Trainium2 Bass kernel for nn_Encoder_47167330845225.

Three embedding+LSTM encoders (source-comment, commit-msg, issue) + scalar
merge + final projection.  Data-parallel over the PR batch (B=64) across 8
NeuronCores; embedding tables and weights replicated.

v2 design (vs v1 which precomputed X = emb@WihT through DRAM):
  - x-projection fused into the recurrence PSUM: gathered embedding chunks
    live in SBUF and Wih matmuls accumulate into the same z banks as the
    recurrence.  No X DRAM bounce, no phase-A PSUM->SBUF copy traffic.
  - recurrence weights in split-fp8 (DoubleRow perf mode, 2 fp8 values per
    PE row):  Whh ~= fp8(64*W)/64 + fp8(1024*(W - hi/64))/1024, consumed
    against two fp8 copies of the state (32*h and 2*h).  All products land
    at a common 2048*z scale in PSUM; Wih is pre-scaled by 2048 in bf16 so
    the x-path accumulates in the same domain.  ACT dequantizes with
    scale=1/2048.  Measured end-to-end ~0.7% rel err vs fp32 reference.
  - gate order per step is bank-major [g | i..f | f..o] in 3 PSUM banks so
    one strided ACT does all 12 sigmoid m-tiles, one ACT does tanh(g)
    (Sigmoid+Tanh live in the same ACT LUT set - no table swaps), one ACT
    does tanh(c).
  - steady state interleaves sc+cm (+is every other round) so the tensor
    engine never waits on the gate chain; after cm ends, sc is split into
    two 40-wide half-streams to keep two streams in flight.
  - gathers run on 4 SWDGE queues with multi-chunk prefetch; Pool engine
    does nothing else so descriptor generation never blocks elementwise.
"""

import math
import os

import numpy as np
import ml_dtypes

BF16 = ml_dtypes.bfloat16
FP8 = ml_dtypes.float8_e4m3          # dt.float8e4
P = 128
V, H, E = 32000, 512, 256
G = 4 * H                            # 2048 gate rows
B, NCOM, LSC, LCM, LIS = 64, 10, 128, 64, 32
NCORES = 8
BPC = B // NCORES                    # 8 PRs per core
NSEQ = BPC * NCOM                    # 80 commit sequences per core
MT = G // P                          # 16 m-tiles
KH = H // P                          # 4 k-tiles over H
KE = E // P                          # 2 k-tiles over E

# (name, T, Nb, chunk_steps, n_chunk_bufs, gather queues)
# chunk_steps * Nb is capped at 640 tokens per dma_gather: 1280-token
# gathers blow the SWDGE descriptor ring and wedge the exec unit
# (NRT_EXEC_UNIT_UNRECOVERABLE) even though CoreSim accepts them.
CHAINS = {
    "sc": (LSC, NSEQ, 8, 8, (0,)),
    "cm": (LCM, NSEQ, 8, 6, (0,)),
    "is": (LIS, BPC, 32, 1, (0,)),
}
_DEBUG = int(os.environ.get("BASSK_DEBUG", "0"))
# bisection flags: NOFP8 -> bf16 recurrence; SPLITACT -> per-bank sigmoid ACTs
_NOFP8 = int(os.environ.get("BASSK_NOFP8", "0"))
_SPLITACT = int(os.environ.get("BASSK_SPLITACT", "0"))
_NOIS = int(os.environ.get("BASSK_NOIS", "0"))      # skip is-chain (runtime probe)
_NOTAIL = int(os.environ.get("BASSK_NOTAIL", "0"))  # skip phase-2 (runtime probe)
_S8 = int(os.environ.get("BASSK_S8", "0"))          # 640-token gather chunks
_NOTANH = int(os.environ.get("BASSK_NOTANH", "0"))  # tanh via 2*sigmoid(2x)-1
_BATCHK = int(os.environ.get("BASSK_BATCHK", "0"))  # batch DR kinds across banks

# bank-major gate order g,i,f,o (pytorch order is i,f,g,o).  g first so
# tanh(g) can be computed while the sigmoid banks' matmuls still run.
_GPERM = np.r_[2 * H:3 * H, 0:H, H:2 * H, 3 * H:4 * H]

_CACHE = {}


class _Layout:
    """Placement of the 16 z m-tiles into PSUM banks for one stream."""

    def __init__(self, tag, w, nbanks, bank_ms):
        self.tag = tag
        self.w = w
        self.nbanks = nbanks
        self.bank_ms = bank_ms          # bank -> list of m-tile indices
        self.col = {}                   # m -> (bank, col0)
        for b, ms in enumerate(bank_ms):
            for i, m in enumerate(ms):
                self.col[m] = (b, i * w)


def _mk_layouts():
    # phase1: 3 banks. bank0: g(m0-3); bank1: m4-9 (i + f/2); bank2: m10-15.
    l_a = _Layout("zA", NSEQ, 3, [[0, 1, 2, 3], [4, 5, 6, 7, 8, 9],
                                  [10, 11, 12, 13, 14, 15]])
    l_b = _Layout("zB", NSEQ, 3, [[0, 1, 2, 3], [4, 5, 6, 7, 8, 9],
                                  [10, 11, 12, 13, 14, 15]])
    # is: everything in one bank (16 m-tiles x 8 cols = 128 <= 512)
    l_is = _Layout("zIS", BPC, 1, [list(range(16))])
    # tail half-streams: 2 banks. bank0: g; bank1: i,f,o (12 x 40 = 480)
    l_t0 = _Layout("zT0", NSEQ // 2, 2, [[0, 1, 2, 3], list(range(4, 16))])
    l_t1 = _Layout("zT1", NSEQ // 2, 2, [[0, 1, 2, 3], list(range(4, 16))])
    return l_a, l_b, l_is, l_t0, l_t1


def _emit(tc, dram, scratch, has_bias):
    import concourse.bass as bass
    import concourse.mybir as mybir
    from contextlib import ExitStack

    dt = mybir.dt
    A = mybir.ActivationFunctionType
    OP = mybir.AluOpType
    DR = mybir.MatmulPerfMode.DoubleRow
    nc = tc.nc
    INV = 1.0 / 2048.0

    with ExitStack() as ctx:
        const = ctx.enter_context(tc.tile_pool(name="const", bufs=1))

        # ---- persistent SBUF: weights, indices, states ----
        whi_sb, wlo_sb, wih_sb, idx_sb, whh_sb = {}, {}, {}, {}, {}
        h_sb, c_sb, h32_sb, h2_sb, b_sb = {}, {}, {}, {}, {}
        for name, (T, Nb, S, NBUF, QS) in CHAINS.items():
            if _NOFP8:
                whh = const.tile([P, KH, G], dt.bfloat16, tag=f"whh_{name}")
                nc.sync.dma_start(whh[:], dram[f"whh_{name}"].rearrange("(k p) g -> p k g", p=P))
                whh_sb[name] = whh
            else:
                whi = const.tile([P, KH, G], dt.float8e4, tag=f"whi_{name}")
                nc.sync.dma_start(whi[:], dram[f"whi_{name}"].rearrange("(k p) g -> p k g", p=P))
                whi_sb[name] = whi
                wlo = const.tile([P, KH, G], dt.float8e4, tag=f"wlo_{name}")
                nc.sync.dma_start(wlo[:], dram[f"wlo_{name}"].rearrange("(k p) g -> p k g", p=P))
                wlo_sb[name] = wlo
            wih = const.tile([P, KE, G], dt.bfloat16, tag=f"wih_{name}")
            nc.sync.dma_start(wih[:], dram[f"wih_{name}"].rearrange("(k p) g -> p k g", p=P))
            wih_sb[name] = wih
            ntok = T * Nb
            ix = const.tile([P, ntok // 16], dt.int16, tag=f"idx_{name}")
            nc.sync.dma_start(ix[:], dram[f"idx_{name}"])
            idx_sb[name] = ix
            if has_bias:
                bb = const.tile([1, G], dt.float32, tag=f"bias_{name}")
                nc.sync.dma_start(bb[:], dram[f"bias_{name}"])
                b_sb[name] = bb
            h = const.tile([P, KH, Nb], dt.bfloat16, tag=f"h_{name}")
            nc.vector.memset(h[:], 0.0)
            h_sb[name] = h
            c = const.tile([P, KH, Nb], dt.bfloat16, tag=f"c_{name}")
            nc.vector.memset(c[:], 0.0)
            c_sb[name] = c
            h32 = const.tile([P, KH, Nb], dt.float8e4, tag=f"h32_{name}")
            nc.vector.memset(h32[:], 0.0)
            h32_sb[name] = h32
            h2 = const.tile([P, KH, Nb], dt.float8e4, tag=f"h2_{name}")
            nc.vector.memset(h2[:], 0.0)
            h2_sb[name] = h2
        ones_sb = None
        if has_bias:
            ones_sb = const.tile([1, NSEQ], dt.bfloat16, tag="ones")
            nc.vector.memset(ones_sb[:], 1.0)

        wm_sb = const.tile([P, KH, 4], dt.bfloat16, tag="wm")
        nc.sync.dma_start(wm_sb[:], dram["wm"].rearrange("(k p) c -> p k c", p=P))
        bm_sb = const.tile([1, 2], dt.float32, tag="bm")
        nc.sync.dma_start(bm_sb[:], dram["bm"])
        wfm_sb = const.tile([P, 2, H], dt.bfloat16, tag="wfm")
        nc.sync.dma_start(wfm_sb[:], dram["wf_m"].rearrange("c p m -> p c m"))
        wfh_sb = const.tile([P, 2, KH, H], dt.bfloat16, tag="wfh")
        nc.sync.dma_start(wfh_sb[:], dram["wf_h"].rearrange("c (k p) m -> p c k m", p=P))
        bf_sb = const.tile([P, KH, 2], dt.float32, tag="bf")
        nc.sync.dma_start(bf_sb[:], dram["bf"].rearrange("(m p) c -> p m c", p=P))

        # ---- gather machinery: chunk ci of chain covers steps
        # [ci*S, (ci+1)*S); tile slot rotates through NBUF buffers ----
        chunk_pool = ctx.enter_context(tc.tile_pool(name="chunks", bufs=1))
        chunk_tiles = {name: {} for name in CHAINS}

        def emit_gather(name, ci):
            T, Nb, S, NBUF, QS = CHAINS[name]
            gch = S * Nb
            emb = chunk_pool.tile([P, KE, gch], dt.bfloat16,
                                  tag=f"emb_{name}", bufs=NBUF,
                                  name=f"emb_{name}_{ci}")
            chunk_tiles[name][ci] = emb
            nc.gpsimd.dma_gather(
                out_ap=emb[:],
                in_ap=dram[f"tab_{name}"][:, :],
                idxs_ap=idx_sb[name][:, ci * (gch // 16):(ci + 1) * (gch // 16)],
                num_idxs=gch,
                num_idxs_reg=gch,
                elem_size=E,
                transpose=True,
                queue_num=0,
            )

        # prologue prefetch: first NBUF chunks of each chain, interleaved so
        # the chunks needed first are gathered first.
        prologue = []
        for i in range(max(nb for (_, _, _, nb, _) in CHAINS.values())):
            for name, (T, Nb, S, NBUF, QS) in CHAINS.items():
                nchunks = (T + S - 1) // S
                if i < min(NBUF, nchunks):
                    prologue.append((name, i))
        for name, ci in prologue:
            emit_gather(name, ci)

        def maybe_refill(r):
            # emit gather for chunk ci of sc once its buffer slot frees:
            # slot (ci mod NBUF) is free after round (ci-NBUF+1)*S - 1.
            for name, (T, Nb, S, NBUF, QS) in CHAINS.items():
                nchunks = (T + S - 1) // S
                for ci in range(NBUF, nchunks):
                    if (ci - NBUF + 1) * S == r + 1:
                        emit_gather(name, ci)

        gate_pool = ctx.enter_context(tc.tile_pool(name="gates", bufs=2))

        def step(zpool, L, name, t, j0):
            """One LSTM step for columns [j0, j0+L.w) of chain `name`."""
            T, Nb, S, NBUF, QS = CHAINS[name]
            w = L.w
            z = zpool.tile([P, L.nbanks, 512], dt.float32, tag=L.tag)
            emb = chunk_tiles[name][t // S]
            xoff = (t % S) * Nb + j0
            wih = wih_sb[name]
            h32 = h32_sb[name]
            h2 = h2_sb[name]

            def _nsub(kind):
                if kind == 0:
                    return KH if _NOFP8 else KH // 2
                if kind == 1:
                    return 0 if _NOFP8 else KH // 2
                if kind == 2:
                    return KE
                return 1

            kinds = range(4 if has_bias else 3)
            seq = []
            if _BATCHK:
                for kind in kinds:
                    for b in range(L.nbanks):
                        for m in L.bank_ms[b]:
                            seq += [(kind, b, m, s) for s in range(_nsub(kind))]
            else:
                for b in range(L.nbanks):
                    for kind in kinds:
                        for m in L.bank_ms[b]:
                            seq += [(kind, b, m, s) for s in range(_nsub(kind))]
            first, last = {}, {}
            for i, (kind, b, m, s) in enumerate(seq):
                first.setdefault(b, i)
                last[b] = i

            def emit_gg():
                gg = gate_pool.tile([P, KH, w], dt.bfloat16, tag=f"gg_{L.tag}")
                gin = z[:, 0, 0:4 * w].rearrange("p (k j) -> p k j", j=w)
                if _NOTANH:
                    g32 = gate_pool.tile([P, KH, w], dt.float32, tag=f"g32_{L.tag}")
                    nc.scalar.activation(g32[:], gin, A.Sigmoid, scale=2.0 * INV)
                    nc.vector.tensor_scalar(gg[:], g32[:], 2.0, -1.0, OP.mult, OP.add)
                else:
                    nc.scalar.activation(gg[:], gin, A.Tanh, scale=INV)
                return gg

            gg = None
            for i, (kind, b, m, s) in enumerate(seq):
                _, col = L.col[m]
                out = z[:, b, col:col + w]
                st, sp = (i == first[b]), (i == last[b])
                if kind == 0 and _NOFP8:
                    nc.tensor.matmul(
                        out, lhsT=whh_sb[name][:, s, m * P:(m + 1) * P],
                        rhs=h_sb[name][:, s, j0:j0 + w],
                        start=st, stop=sp, skip_group_check=True)
                elif kind == 0:
                    nc.tensor.matmul(
                        out, lhsT=whi_sb[name][:, 2 * s:2 * s + 2, m * P:(m + 1) * P],
                        rhs=h32[:, 2 * s:2 * s + 2, j0:j0 + w], perf_mode=DR,
                        start=st, stop=sp, skip_group_check=True)
                elif kind == 1:
                    nc.tensor.matmul(
                        out, lhsT=wlo_sb[name][:, 2 * s:2 * s + 2, m * P:(m + 1) * P],
                        rhs=h2[:, 2 * s:2 * s + 2, j0:j0 + w], perf_mode=DR,
                        start=st, stop=sp, skip_group_check=True)
                elif kind == 2:
                    nc.tensor.matmul(
                        out, lhsT=wih[:, s, m * P:(m + 1) * P],
                        rhs=emb[:, s, xoff:xoff + w],
                        start=st, stop=sp, skip_group_check=True)
                else:
                    nc.tensor.matmul(
                        out, lhsT=b_sb[name][0:1, m * P:(m + 1) * P],
                        rhs=ones_sb[0:1, :w],
                        start=st, stop=sp, skip_group_check=True)
                if i == last[0] and gg is None:
                    gg = emit_gg()   # tanh(g) as soon as the g bank completes

            # sigmoid over the i/f/o banks in one strided ACT.
            nsig = 12 * w
            sig = gate_pool.tile([P, nsig], dt.bfloat16, tag=f"sig_{L.tag}")
            if L.nbanks == 3 and _SPLITACT:
                nc.scalar.activation(sig[:, 0:6 * w], z[:, 1, 0:6 * w], A.Sigmoid, scale=INV)
                nc.scalar.activation(sig[:, 6 * w:12 * w], z[:, 2, 0:6 * w], A.Sigmoid, scale=INV)
            elif L.nbanks == 3:
                nc.scalar.activation(
                    sig[:].rearrange("p (a b) -> p a b", a=2),
                    z[:, 1:3, 0:6 * w], A.Sigmoid, scale=INV)
            elif L.nbanks == 2:
                nc.scalar.activation(sig[:], z[:, 1, 0:nsig], A.Sigmoid, scale=INV)
            else:
                nc.scalar.activation(sig[:], z[:, 0, 4 * w:16 * w], A.Sigmoid, scale=INV)
            ig = sig[:, 0 * 4 * w:1 * 4 * w].rearrange("p (k j) -> p k j", j=w)
            fg = sig[:, 1 * 4 * w:2 * 4 * w].rearrange("p (k j) -> p k j", j=w)
            og = sig[:, 2 * 4 * w:3 * 4 * w].rearrange("p (k j) -> p k j", j=w)

            cs = c_sb[name][:, :, j0:j0 + w]
            hs = h_sb[name][:, :, j0:j0 + w]
            tmp = gate_pool.tile([P, KH, w], dt.bfloat16, tag=f"tmp_{L.tag}")
            tch = gate_pool.tile([P, KH, w], dt.bfloat16, tag=f"tch_{L.tag}")
            nc.vector.tensor_mul(tmp[:], ig, gg[:])
            nc.vector.tensor_mul(cs, fg, cs)
            nc.vector.tensor_add(cs, cs, tmp[:])
            if _NOTANH:
                t32 = gate_pool.tile([P, KH, w], dt.float32, tag=f"t32_{L.tag}")
                nc.scalar.activation(t32[:], cs, A.Sigmoid, scale=2.0)
                nc.vector.tensor_scalar(tch[:], t32[:], 2.0, -1.0, OP.mult, OP.add)
            else:
                nc.scalar.activation(tch[:], cs, A.Tanh)
            nc.vector.tensor_mul(hs, og, tch[:])
            if not _NOFP8:
                nc.vector.tensor_scalar(h32_sb[name][:, :, j0:j0 + w], hs, 32.0, None, OP.mult)
                nc.vector.tensor_scalar(h2_sb[name][:, :, j0:j0 + w], hs, 2.0, None, OP.mult)

        l_a, l_b, l_is, l_t0, l_t1 = _mk_layouts()

        # ---- phase 1: rounds 0..63 -- sc + cm (+ is every other round) ----
        with tc.tile_pool(name="z1", bufs=1, space="PSUM") as zpool:
            for r in range(LCM):
                step(zpool, l_a, "sc", r, 0)
                if r % 2 == 0 and not _NOIS:
                    step(zpool, l_is, "is", r // 2, 0)
                step(zpool, l_b, "cm", r, 0)
                maybe_refill(r)

        # ---- phase 2: rounds 64..127 -- sc as two half-streams ----
        with tc.tile_pool(name="z2", bufs=1, space="PSUM") as zpool:
            for r in range(LCM, LSC):
                if _NOTAIL:
                    break
                step(zpool, l_t0, "sc", r, 0)
                step(zpool, l_t1, "sc", r, NSEQ // 2)
                maybe_refill(r)

        if _DEBUG:
            for name in CHAINS:
                nc.sync.dma_start(dram[f"dbg_h_{name}"][:], h_sb[name][:])
                nc.sync.dma_start(dram[f"dbg_c_{name}"][:], c_sb[name][:])

        # ---- phase 3: merge + final projection (same as v1) ----
        with tc.tile_pool(name="fin", bufs=1) as fin, \
             tc.tile_pool(name="fpsum", bufs=2, space="PSUM") as fp:
            for side, st1, st2, st_is in (
                    (0, h_sb["sc"], h_sb["cm"], h_sb["is"]),
                    (1, c_sb["sc"], c_sb["cm"], c_sb["is"])):
                # hm[j] = hcat[j] . wm  over both halves
                mm = fp.tile([1, NSEQ], dt.float32, tag="mg")
                for half, st in ((0, st1), (1, st2)):
                    for k in range(KH):
                        col = 2 * side + half
                        nc.tensor.matmul(
                            mm[:], lhsT=wm_sb[:, k, col:col + 1], rhs=st[:, k, :],
                            start=(half == 0 and k == 0),
                            stop=(half == 1 and k == KH - 1),
                            skip_group_check=True)
                hm_bf = fin.tile([1, NSEQ], dt.bfloat16, tag=f"hm{side}")
                nc.vector.tensor_scalar(
                    hm_bf[:], mm[:], bm_sb[0:1, side:side + 1], None, OP.add)
                # reshape [80] -> [10, 8] via DRAM bounce; zero-pad to 128 rows
                nc.sync.dma_start(scratch[side][None, :], hm_bf[0:1, :])
                hmT = fin.tile([P, BPC], dt.bfloat16, tag=f"hmT{side}")
                nc.vector.memset(hmT[:], 0.0)
                nc.sync.dma_start(
                    hmT[:NCOM, :], scratch[side].rearrange("(p n) -> n p", n=NCOM))
                out_sb = fin.tile([P, KH, BPC], dt.float32, tag=f"out{side}")
                for m in range(KH):
                    pf = fp.tile([P, BPC], dt.float32, tag="fin")
                    nc.tensor.matmul(
                        pf[:], lhsT=wfm_sb[:, side, m * P:(m + 1) * P], rhs=hmT[:],
                        start=True, stop=False, skip_group_check=True)
                    for k in range(KH):
                        nc.tensor.matmul(
                            pf[:], lhsT=wfh_sb[:, side, k, m * P:(m + 1) * P],
                            rhs=st_is[:, k, :],
                            start=False, stop=(k == KH - 1),
                            skip_group_check=True)
                    nc.scalar.activation(
                        out_sb[:, m, :], pf[:], A.Identity,
                        bias=bf_sb[:, m, side:side + 1])
                nc.sync.dma_start(dram["ho" if side == 0 else "co"][:], out_sb[:])


def _build(has_bias):
    import concourse.mybir as mybir
    import concourse.tile as tile
    from concourse import bacc

    dt = mybir.dt
    nc = bacc.Bacc("TRN2", target_bir_lowering=False, debug=False,
                   num_devices=NCORES)
    dram = {}
    for name, (T, Nb, S, NBUF, QS) in CHAINS.items():
        dram[f"tab_{name}"] = nc.dram_tensor(f"tab_{name}", [V, E], dt.bfloat16, kind="ExternalInput").ap()
        if _NOFP8:
            dram[f"whh_{name}"] = nc.dram_tensor(f"whh_{name}", [H, G], dt.bfloat16, kind="ExternalInput").ap()
        else:
            dram[f"whi_{name}"] = nc.dram_tensor(f"whi_{name}", [H, G], dt.float8e4, kind="ExternalInput").ap()
            dram[f"wlo_{name}"] = nc.dram_tensor(f"wlo_{name}", [H, G], dt.float8e4, kind="ExternalInput").ap()
        dram[f"wih_{name}"] = nc.dram_tensor(f"wih_{name}", [E, G], dt.bfloat16, kind="ExternalInput").ap()
        if has_bias:
            dram[f"bias_{name}"] = nc.dram_tensor(f"bias_{name}", [1, G], dt.float32, kind="ExternalInput").ap()
        dram[f"idx_{name}"] = nc.dram_tensor(f"idx_{name}", [P, T * Nb // 16], dt.int16, kind="ExternalInput").ap()
    dram["wm"] = nc.dram_tensor("wm", [H, 4], dt.bfloat16, kind="ExternalInput").ap()
    dram["bm"] = nc.dram_tensor("bm", [1, 2], dt.float32, kind="ExternalInput").ap()
    dram["wf_m"] = nc.dram_tensor("wf_m", [2, P, H], dt.bfloat16, kind="ExternalInput").ap()
    dram["wf_h"] = nc.dram_tensor("wf_h", [2, H, H], dt.bfloat16, kind="ExternalInput").ap()
    dram["bf"] = nc.dram_tensor("bf", [H, 2], dt.float32, kind="ExternalInput").ap()
    dram["ho"] = nc.dram_tensor("ho", [P, KH, BPC], dt.float32, kind="ExternalOutput").ap()
    dram["co"] = nc.dram_tensor("co", [P, KH, BPC], dt.float32, kind="ExternalOutput").ap()
    if _DEBUG:
        for name, (T, Nb, S, NBUF, QS) in CHAINS.items():
            dram[f"dbg_h_{name}"] = nc.dram_tensor(f"dbg_h_{name}", [P, KH, Nb], dt.bfloat16, kind="ExternalOutput").ap()
            dram[f"dbg_c_{name}"] = nc.dram_tensor(f"dbg_c_{name}", [P, KH, Nb], dt.bfloat16, kind="ExternalOutput").ap()

    scratch = [nc.dram_tensor(f"hmsc{i}", [NSEQ], dt.bfloat16, kind="Internal").ap() for i in range(2)]

    with tile.TileContext(nc) as tc:
        _emit(tc, dram, scratch, has_bias)
    nc.compile()
    return nc


def _prep_inputs(inputs):
    """Build the 8 per-core input maps from full-size inputs."""
    comments = np.asarray(inputs["comments"]).astype(np.int32)
    cm = np.asarray(inputs["cm"]).astype(np.int32)
    issue = np.asarray(inputs["issue"]).astype(np.int32)

    def bf(x):
        return np.ascontiguousarray(np.asarray(x).astype(BF16))

    has_bias = any(
        np.any(np.asarray(inputs[b]) != 0) for b in ("b_sc", "b_cm", "b_is"))

    shared = {}
    for name, src in (("sc", "emb_sc"), ("cm", "emb_cm"), ("is", "emb_is")):
        shared[f"tab_{name}"] = bf(inputs[src])
    for name, whh, wih, b in (("sc", "Whh_sc", "Wih_sc", "b_sc"),
                              ("cm", "Whh_cm", "Wih_cm", "b_cm"),
                              ("is", "Whh_is", "Wih_is", "b_is")):
        Wp = np.asarray(inputs[whh], np.float32)[_GPERM].T      # [H, G]
        hi = (64.0 * Wp).astype(FP8)
        resid = Wp - hi.astype(np.float32) / 64.0
        lo = (1024.0 * resid).astype(FP8)
        shared[f"whi_{name}"] = np.ascontiguousarray(hi)
        shared[f"wlo_{name}"] = np.ascontiguousarray(lo)
        shared[f"whh_{name}"] = bf(2048.0 * Wp)
        Up = np.asarray(inputs[wih], np.float32)[_GPERM].T      # [E, G]
        shared[f"wih_{name}"] = bf(2048.0 * Up)
        if has_bias:
            shared[f"bias_{name}"] = np.ascontiguousarray(
                2048.0 * np.asarray(inputs[b], np.float32)[_GPERM][None, :])
    wm = np.stack([np.asarray(inputs["Wmh"])[0, :H],
                   np.asarray(inputs["Wmh"])[0, H:],
                   np.asarray(inputs["Wmc"])[0, :H],
                   np.asarray(inputs["Wmc"])[0, H:]], axis=1)   # [H, 4]
    shared["wm"] = bf(wm)
    shared["bm"] = np.array([[inputs["bmh"][0], inputs["bmc"][0]]], dtype=np.float32)
    wf_m = np.zeros((2, P, H), np.float32)
    wf_h = np.zeros((2, H, H), np.float32)
    for i, w in enumerate(("Wfh", "Wfc")):
        WT = np.asarray(inputs[w]).T                    # [522, 512]
        wf_m[i, :NCOM] = WT[:NCOM]
        wf_h[i] = WT[NCOM:]
    shared["wf_m"] = bf(wf_m)
    shared["wf_h"] = bf(wf_h)
    shared["bf"] = np.ascontiguousarray(
        np.stack([inputs["bfh"], inputs["bfc"]], axis=1).astype(np.float32))

    def wrap16(flat):
        # dma_gather index layout: idx i -> [i % 16, i // 16], int16,
        # replicated over all 128 partitions (8 gpsimd channels x 16).
        w = flat.reshape(-1, 16).T.astype(np.int16)     # [16, n/16]
        return np.ascontiguousarray(np.tile(w, (P // 16, 1)))

    in_maps = []
    for c in range(NCORES):
        m = dict(shared)
        prs = slice(c * BPC, (c + 1) * BPC)
        # time-major token ids: token f = t*Nb + j, j = pr_local*NCOM + ncom
        sc = comments[prs].reshape(NSEQ, LSC)
        m["idx_sc"] = wrap16(sc.T.reshape(-1))
        cmv = cm[prs].reshape(NSEQ, LCM)
        m["idx_cm"] = wrap16(cmv.T.reshape(-1))
        isv = issue[prs]                                # [8, T]
        m["idx_is"] = wrap16(isv.T.reshape(-1))
        in_maps.append(m)
    return in_maps, has_bias


def kernel(**inputs):
    from concourse.bass_utils import run_bass_kernel_spmd

    in_maps, has_bias = _prep_inputs(inputs)
    if "nc" not in _CACHE:
        _CACHE["nc"] = _build(has_bias)
    res = run_bass_kernel_spmd(_CACHE["nc"], in_maps, core_ids=list(range(NCORES)))
    h = np.zeros((B, H), np.float32)
    c = np.zeros((B, H), np.float32)
    for ci, r in enumerate(res.results):
        # ho [128, 4, 8]: ho[p, k, j] = h[8*ci + j, 128*k + p]
        h[ci * BPC:(ci + 1) * BPC] = r["ho"].transpose(2, 1, 0).reshape(BPC, H)
        c[ci * BPC:(ci + 1) * BPC] = r["co"].transpose(2, 1, 0).reshape(BPC, H)
    return h[None], c[None]
